# revision 17
# baseline (speedup 1.0000x reference)
"""Trainium2 Bass kernel for DropChannel (topk channel masking).

Math (per sample):
    score_c = sum_hw x[hw, c]                        (1/HW cancels in the key)
    lk_c    = ln(r_c) / score_c                      (log key; order-preserving)
    gcnt_i  = #{c : lk_c > lk_i}                     (strictly-greater count)
    sel_i   = gcnt_i < C - M                         (matches thr = sort(key)[C-M]; sel = key >= thr,
                                                      including tie behaviour)
    alpha   = sum(S) / sum(S * sel)
    out     = x * (sel & (u < P)) * alpha

Sharding: pure data parallel, N=32 samples -> 8 cores x 4 samples.

v2 schedule (per core, 4 samples, each [4096, 1024] f32):
  - x streamed in 2-tile f32 chunks [128, 2048] on the SP HWDGE queue;
    each chunk feeds (a) fp32 PE ones-matmuls accumulating the channel
    score sums in PSUM and (b) an ACT-engine Copy that materializes a
    resident bf16 replica.  The f32 buffer then recycles immediately,
    so prefetch depth is bounded by the cheap bf16 pool, not f32.
  - output is written to HBM in bf16 (rel-err ~2^-9 per rounding, far
    inside the 2e-2 gate) halving store traffic: 96 MiB/core total.
  - gcnt by free-axis accumulation: tensor_scalar(is_gt) with accum_out
    against a broadcast lk row, all in (p k) column layout; no PE
    matmuls and no [128,512] compare tensors to store.
  - alpha via gpsimd.partition_all_reduce (no row-layout round trip).
  - pass 2 is a bf16 TensorTensor (DVE 2x mode) against the
    alpha-scaled broadcast mask, in place in the bf16 replica.
  - queues: loads own the SP HWDGE queue; bf16 conversions + fast mid
    DMAs ride ACT; the mask row DMA (which waits on the compare
    chain), the broadcasts, and all output stores ride the gpsimd
    SWDGE queue so they never block the ACT conversion stream or the
    SP prefetch stream.
"""

import numpy as np
from contextlib import ExitStack

import concourse.bacc as bacc
import concourse.tile as tile
from concourse import mybir
from concourse import bass_isa
from concourse.bass_utils import run_bass_kernel_spmd

N, HW, C = 32, 4096, 1024
NCORES = 8
NS = N // NCORES          # samples per core
P = 128                   # partitions
CK = C // P               # 8 channels per partition in (p k) layout
NKEEP = C - int(0.5 * C)  # gcnt threshold: keep rows with gcnt < 512
PKEEP = 0.9
HALF = 512                # matmul free-dim limit (one PSUM bank)
CT = 2                    # tiles per DMA chunk
CW = CT * C               # chunk free width (f32 elems)

f32 = mybir.dt.float32
bf16 = mybir.dt.bfloat16
ALU = mybir.AluOpType
ACTF = mybir.ActivationFunctionType
AXIS = mybir.AxisListType

# The score sums stay fp32 end to end: the selection boundary sits only
# ~2e-5 (relative) from the threshold, so bf16 anywhere in the score /
# log-key path flips channels.  The data path (x replica, mask, output)
# is bf16: three roundings of <=2^-9 each, ~0.6% worst case.


def emit(tc, o, x, r, u, ns, hw, xbufs, xbbufs):
    nc = tc.nc
    nt = hw // P              # 32 tiles
    nk = nt // CT             # chunks per sample
    # chunk view: hw = (k*CT + t)*P + p ; free dims [p][t][c]
    xk = x.rearrange("s (k t p) c -> s k p t c", t=CT, p=P)
    ok = o.rearrange("s (k t p) c -> s k p t c", t=CT, p=P)
    rck = r.rearrange("s (p k) -> s p k", k=CK)
    uck = u.rearrange("s (p k) -> s p k", k=CK)

    with ExitStack() as ctx:
        xpool = ctx.enter_context(tc.tile_pool(name="xpool", bufs=xbufs))
        xbpool = ctx.enter_context(tc.tile_pool(name="xbpool", bufs=xbbufs))
        tqpool = ctx.enter_context(tc.tile_pool(name="tqpool", bufs=3))
        bcpool = ctx.enter_context(tc.tile_pool(name="bcpool", bufs=2))
        mkpool = ctx.enter_context(tc.tile_pool(name="mkpool", bufs=2))
        scpool = ctx.enter_context(tc.tile_pool(name="scpool", bufs=2))
        rows = ctx.enter_context(tc.tile_pool(name="rows", bufs=2))
        consts = ctx.enter_context(tc.tile_pool(name="consts", bufs=1))
        ps_s = ctx.enter_context(tc.tile_pool(name="ps_s", bufs=2, space="PSUM"))

        ones_col = consts.tile([P, 1], f32)
        nc.vector.memset(ones_col, 1.0)
        ones_bc = consts.tile([P, C], bf16)
        nc.vector.memset(ones_bc, 1.0)

        def pass1(s):
            """Stream chunks of sample s: PE accumulates score sums in
            PSUM, ACT writes the resident bf16 replica, DVE pair-adds
            (fp32 PE matmuls lower to 2 half-speed passes each, so
            halving the matmul count is worth one f32 add per chunk).
            Also the x-independent precompute: ln(r), bernoulli gate."""
            lnr_cols = rows.tile([P, CK], f32, tag="lnr_cols")
            nc.sync.dma_start(out=lnr_cols, in_=rck[s])
            nc.scalar.activation(lnr_cols, lnr_cols, ACTF.Ln)
            rng_cols = rows.tile([P, CK], f32, tag="rng_cols")
            nc.sync.dma_start(out=rng_cols, in_=uck[s])
            nc.vector.tensor_scalar(
                rng_cols, rng_cols, PKEEP, None, op0=ALU.is_lt
            )
            ps_score = ps_s.tile([1, C], f32, tag="ps_score")
            xbs = []
            for k in range(nk):
                xc = xpool.tile([P, CW], f32, tag="xc")
                nc.sync.dma_start(out=xc, in_=xk[s, k])
                xb = xbpool.tile([P, CW], bf16, tag="xb")
                nc.scalar.copy(xb, xc)
                xbs.append(xb)
                acc = tqpool.tile([P, C], f32, tag="acc")
                nc.vector.tensor_add(acc, xc[:, 0:C], xc[:, C:CW])
                for h in range(2):
                    nc.tensor.matmul(
                        ps_score[:, h * HALF:(h + 1) * HALF],
                        lhsT=ones_col,
                        rhs=acc[:, h * HALF:(h + 1) * HALF],
                        start=(k == 0),
                        stop=(k == nk - 1),
                    )
            return lnr_cols, rng_cols, ps_score, xbs

        def mid(s, lnr_cols, rng_cols, ps_score):
            # ---- mid: selection mask + alpha, column layout ----
            s_row = rows.tile([1, C], f32, tag="s_row", bufs=1)
            nc.vector.tensor_copy(s_row, ps_score)
            s_cols = rows.tile([P, CK], f32, tag="s_cols", bufs=1)
            nc.scalar.dma_start(out=s_cols, in_=s_row)
            recip_cols = rows.tile([P, CK], f32, tag="recip_cols", bufs=1)
            nc.vector.reciprocal(recip_cols, s_cols)
            lk_cols = rows.tile([P, CK], f32, tag="lk_cols", bufs=1)
            nc.vector.tensor_mul(lk_cols, lnr_cols, recip_cols)
            lk_row = rows.tile([1, C], f32, tag="lk_row", bufs=1)
            nc.scalar.dma_start(out=lk_row, in_=lk_cols)

            # gcnt_i = #{c : lk_c > lk_i}: compare the full broadcast-lk
            # row against per-partition lk scalars, accumulating along
            # the free axis -> gcnt directly in (p k) layout.
            # accum_out via STT: tensor_scalar's accum path writes zeros
            # on hardware, STT's is the baseline-proven one.
            gcnt_cols = rows.tile([P, CK], f32, tag="gcnt_cols", bufs=1)
            scrap = scpool.tile([P, C], bf16, tag="scrap", bufs=2)
            b_bc = bcpool.tile([P, C], f32, tag="b_bc")
            nc.gpsimd.partition_broadcast(b_bc, lk_row)
            for q in range(CK):
                nc.vector.scalar_tensor_tensor(
                    scrap, b_bc, lk_cols[:, q:q + 1], ones_bc,
                    op0=ALU.is_gt, op1=ALU.mult,
                    accum_out=gcnt_cols[:, q:q + 1],
                )

            # alpha = sum(S) / sum(S * sel), all-reduced across partitions
            stats = rows.tile([P, 2], f32, tag="stats", bufs=1)
            sel8 = rows.tile([P, CK], f32, tag="sel8", bufs=1)
            nc.vector.scalar_tensor_tensor(
                sel8, gcnt_cols, float(NKEEP), s_cols,
                op0=ALU.is_lt, op1=ALU.mult, accum_out=stats[:, 0:1],
            )
            nc.vector.tensor_reduce(stats[:, 1:2], s_cols, axis=AXIS.X, op=ALU.add)
            statsr = rows.tile([P, 2], f32, tag="statsr", bufs=1)
            nc.gpsimd.partition_all_reduce(
                statsr, stats, channels=P, reduce_op=bass_isa.ReduceOp.add
            )
            alpha_pp = rows.tile([P, 1], f32, tag="alpha_pp", bufs=1)
            nc.vector.reciprocal(alpha_pp, statsr[:, 0:1])
            nc.vector.tensor_mul(alpha_pp, alpha_pp, statsr[:, 1:2])

            # mask = (sel & rng) * alpha, rounded once to bf16
            mask_cols = rows.tile([P, CK], f32, tag="mask_cols", bufs=1)
            nc.vector.scalar_tensor_tensor(
                mask_cols, gcnt_cols, float(NKEEP), rng_cols,
                op0=ALU.is_lt, op1=ALU.mult,
            )
            maskb_cols = rows.tile([P, CK], bf16, tag="maskb_cols", bufs=1)
            nc.vector.tensor_scalar(
                maskb_cols, mask_cols, alpha_pp, None, op0=ALU.mult
            )
            maskb_row = rows.tile([1, C], bf16, tag="maskb_row", bufs=1)
            nc.gpsimd.dma_start(out=maskb_row, in_=maskb_cols)
            mask_bc = mkpool.tile([P, C], bf16, tag="mask_bc")
            nc.gpsimd.partition_broadcast(mask_bc, maskb_row)
            return mask_bc

        def pass2(s, xbs, mask_bc):
            # ---- pass 2: out = xb * mask, in place (bf16 2x mode) ----
            for k in range(nk):
                for t in range(CT):
                    nc.vector.tensor_mul(
                        xbs[k][:, t * C:(t + 1) * C],
                        xbs[k][:, t * C:(t + 1) * C],
                        mask_bc,
                    )
                nc.gpsimd.dma_start(out=ok[s, k], in_=xbs[k])

        # Software-pipelined emission: mid(s) -> pass1(s+1) -> pass2(s).
        # Keeping pass1(s+1)'s DVE pair-adds AHEAD of pass2(s)'s 32
        # multiplies in the DVE program order lets the f32 chunk pool
        # recycle during mid+pass2, so the SP load stream never stalls
        # at sample boundaries (this was a ~25us/sample DMA valley).
        st = pass1(0)
        for s in range(ns):
            mask_bc = mid(s, st[0], st[1], st[2])
            xbs = st[3]
            if s + 1 < ns:
                st = pass1(s + 1)
            pass2(s, xbs, mask_bc)


def build_nc(ns=NS, hw=HW, xbufs=8, xbbufs=25):
    nc = bacc.Bacc(
        "TRN2", target_bir_lowering=False, debug=False, num_devices=NCORES
    )
    x = nc.dram_tensor("x", [ns, hw, C], f32, kind="ExternalInput").ap()
    r = nc.dram_tensor("r", [ns, C], f32, kind="ExternalInput").ap()
    u = nc.dram_tensor("u", [ns, C], f32, kind="ExternalInput").ap()
    o = nc.dram_tensor("o", [ns, hw, C], bf16, kind="ExternalOutput").ap()
    with tile.TileContext(nc) as tc:
        emit(tc, o, x, r, u, ns, hw, xbufs, xbbufs)
    nc.compile()
    return nc


_cached_nc = None


def kernel(x, r, u):
    global _cached_nc
    if _cached_nc is None:
        _cached_nc = build_nc()
    in_maps = [
        {
            "x": np.ascontiguousarray(x[i * NS:(i + 1) * NS], dtype=np.float32),
            "r": np.ascontiguousarray(r[i * NS:(i + 1) * NS], dtype=np.float32),
            "u": np.ascontiguousarray(u[i * NS:(i + 1) * NS], dtype=np.float32),
        }
        for i in range(NCORES)
    ]
    res = run_bass_kernel_spmd(_cached_nc, in_maps, list(range(NCORES))).results
    return np.concatenate(
        [np.asarray(res[i]["o"], dtype=np.float32) for i in range(NCORES)], axis=0
    )


# revision 18
# speedup vs baseline: 1.0594x; 1.0594x over previous
"""Trainium2 Bass kernel for DropChannel (topk channel masking).

Math (per sample):
    score_c = sum_hw x[hw, c]                        (1/HW cancels in the key)
    lk_c    = ln(r_c) / score_c                      (log key; order-preserving)
    gcnt_i  = #{c : lk_c > lk_i}                     (strictly-greater count)
    sel_i   = gcnt_i < C - M                         (matches thr = sort(key)[C-M]; sel = key >= thr,
                                                      including tie behaviour)
    alpha   = sum(S) / sum(S * sel)
    out     = x * (sel & (u < P)) * alpha

Sharding: pure data parallel, N=32 samples -> 8 cores x 4 samples.

v2 schedule (per core, 4 samples, each [4096, 1024] f32):
  - x streamed in 2-tile f32 chunks [128, 2048] on the SP HWDGE queue;
    each chunk feeds (a) fp32 PE ones-matmuls accumulating the channel
    score sums in PSUM and (b) an ACT-engine Copy that materializes a
    resident bf16 replica.  The f32 buffer then recycles immediately,
    so prefetch depth is bounded by the cheap bf16 pool, not f32.
  - output is written to HBM in bf16 (rel-err ~2^-9 per rounding, far
    inside the 2e-2 gate) halving store traffic: 96 MiB/core total.
  - gcnt by free-axis accumulation: tensor_scalar(is_gt) with accum_out
    against a broadcast lk row, all in (p k) column layout; no PE
    matmuls and no [128,512] compare tensors to store.
  - alpha via gpsimd.partition_all_reduce (no row-layout round trip).
  - pass 2 is a bf16 TensorTensor (DVE 2x mode) against the
    alpha-scaled broadcast mask, in place in the bf16 replica.
  - queues: loads own the SP HWDGE queue; bf16 conversions + fast mid
    DMAs ride ACT; the mask row DMA (which waits on the compare
    chain), the broadcasts, and all output stores ride the gpsimd
    SWDGE queue so they never block the ACT conversion stream or the
    SP prefetch stream.
"""

import numpy as np
from contextlib import ExitStack

import concourse.bacc as bacc
import concourse.tile as tile
from concourse import mybir
from concourse import bass_isa
from concourse.bass_utils import run_bass_kernel_spmd

N, HW, C = 32, 4096, 1024
NCORES = 8
NS = N // NCORES          # samples per core
P = 128                   # partitions
CK = C // P               # 8 channels per partition in (p k) layout
NKEEP = C - int(0.5 * C)  # gcnt threshold: keep rows with gcnt < 512
PKEEP = 0.9
HALF = 512                # matmul free-dim limit (one PSUM bank)
CT = 2                    # tiles per DMA chunk
CW = CT * C               # chunk free width (f32 elems)

f32 = mybir.dt.float32
bf16 = mybir.dt.bfloat16
ALU = mybir.AluOpType
ACTF = mybir.ActivationFunctionType
AXIS = mybir.AxisListType

# The score sums stay fp32 end to end: the selection boundary sits only
# ~2e-5 (relative) from the threshold, so bf16 anywhere in the score /
# log-key path flips channels.  The data path (x replica, mask, output)
# is bf16: three roundings of <=2^-9 each, ~0.6% worst case.


def emit(tc, o, x, r, u, ns, hw, xbufs, xbbufs):
    nc = tc.nc
    nt = hw // P              # 32 tiles
    nk = nt // CT             # chunks per sample
    # chunk view: hw = (k*CT + t)*P + p ; free dims [p][t][c]
    xk = x.rearrange("s (k t p) c -> s k p t c", t=CT, p=P)
    ok = o.rearrange("s (k t p) c -> s k p t c", t=CT, p=P)
    rck = r.rearrange("s (p k) -> s p k", k=CK)
    uck = u.rearrange("s (p k) -> s p k", k=CK)

    with ExitStack() as ctx:
        xpool = ctx.enter_context(tc.tile_pool(name="xpool", bufs=xbufs))
        xbpool = ctx.enter_context(tc.tile_pool(name="xbpool", bufs=xbbufs))
        tqpool = ctx.enter_context(tc.tile_pool(name="tqpool", bufs=3))
        bcpool = ctx.enter_context(tc.tile_pool(name="bcpool", bufs=2))
        mkpool = ctx.enter_context(tc.tile_pool(name="mkpool", bufs=2))
        scpool = ctx.enter_context(tc.tile_pool(name="scpool", bufs=2))
        rows = ctx.enter_context(tc.tile_pool(name="rows", bufs=2))
        consts = ctx.enter_context(tc.tile_pool(name="consts", bufs=1))
        ps_s = ctx.enter_context(tc.tile_pool(name="ps_s", bufs=2, space="PSUM"))

        ones_col = consts.tile([P, 1], f32)
        nc.vector.memset(ones_col, 1.0)
        ones_bc = consts.tile([P, C], bf16)
        nc.vector.memset(ones_bc, 1.0)

        def pass1(s):
            """Stream chunks of sample s: PE accumulates score sums in
            PSUM, ACT writes the resident bf16 replica, DVE pair-adds
            (fp32 PE matmuls lower to 2 half-speed passes each, so
            halving the matmul count is worth one f32 add per chunk).
            Also the x-independent precompute: ln(r), bernoulli gate."""
            lnr_cols = rows.tile([P, CK], f32, tag="lnr_cols")
            nc.sync.dma_start(out=lnr_cols, in_=rck[s])
            nc.scalar.activation(lnr_cols, lnr_cols, ACTF.Ln)
            rng_cols = rows.tile([P, CK], f32, tag="rng_cols")
            nc.sync.dma_start(out=rng_cols, in_=uck[s])
            nc.vector.tensor_scalar(
                rng_cols, rng_cols, PKEEP, None, op0=ALU.is_lt
            )
            ps_score = ps_s.tile([1, C], f32, tag="ps_score")
            xbs = []
            for k in range(nk):
                xc = xpool.tile([P, CW], f32, tag="xc")
                nc.sync.dma_start(out=xc, in_=xk[s, k])
                xb = xbpool.tile([P, CW], bf16, tag="xb")
                nc.scalar.copy(xb, xc)
                xbs.append(xb)
                acc = tqpool.tile([P, C], f32, tag="acc")
                nc.vector.tensor_add(acc, xc[:, 0:C], xc[:, C:CW])
                for h in range(2):
                    nc.tensor.matmul(
                        ps_score[:, h * HALF:(h + 1) * HALF],
                        lhsT=ones_col,
                        rhs=acc[:, h * HALF:(h + 1) * HALF],
                        start=(k == 0),
                        stop=(k == nk - 1),
                    )
            return lnr_cols, rng_cols, ps_score, xbs

        def mid(s, lnr_cols, rng_cols, ps_score):
            # ---- mid: selection mask + alpha, column layout ----
            s_row = rows.tile([1, C], f32, tag="s_row", bufs=1)
            nc.vector.tensor_copy(s_row, ps_score)
            s_cols = rows.tile([P, CK], f32, tag="s_cols", bufs=1)
            nc.scalar.dma_start(out=s_cols, in_=s_row)
            recip_cols = rows.tile([P, CK], f32, tag="recip_cols", bufs=1)
            nc.vector.reciprocal(recip_cols, s_cols)
            lk_cols = rows.tile([P, CK], f32, tag="lk_cols", bufs=1)
            nc.vector.tensor_mul(lk_cols, lnr_cols, recip_cols)
            lk_row = rows.tile([1, C], f32, tag="lk_row", bufs=1)
            nc.scalar.dma_start(out=lk_row, in_=lk_cols)

            # gcnt_i = #{c : lk_c > lk_i}: compare the full broadcast-lk
            # row against per-partition lk scalars, accumulating along
            # the free axis -> gcnt directly in (p k) layout.
            # accum_out via STT: tensor_scalar's accum path writes zeros
            # on hardware, STT's is the baseline-proven one.
            gcnt_cols = rows.tile([P, CK], f32, tag="gcnt_cols", bufs=1)
            scrap = scpool.tile([P, C], bf16, tag="scrap", bufs=2)
            b_bc = bcpool.tile([P, C], f32, tag="b_bc")
            nc.gpsimd.partition_broadcast(b_bc, lk_row)
            for q in range(CK):
                nc.vector.scalar_tensor_tensor(
                    scrap, b_bc, lk_cols[:, q:q + 1], ones_bc,
                    op0=ALU.is_gt, op1=ALU.mult,
                    accum_out=gcnt_cols[:, q:q + 1],
                )

            # alpha = sum(S) / sum(S * sel), all-reduced across partitions
            stats = rows.tile([P, 2], f32, tag="stats", bufs=1)
            sel8 = rows.tile([P, CK], f32, tag="sel8", bufs=1)
            nc.vector.scalar_tensor_tensor(
                sel8, gcnt_cols, float(NKEEP), s_cols,
                op0=ALU.is_lt, op1=ALU.mult, accum_out=stats[:, 0:1],
            )
            nc.vector.tensor_reduce(stats[:, 1:2], s_cols, axis=AXIS.X, op=ALU.add)
            statsr = rows.tile([P, 2], f32, tag="statsr", bufs=1)
            nc.gpsimd.partition_all_reduce(
                statsr, stats, channels=P, reduce_op=bass_isa.ReduceOp.add
            )
            alpha_pp = rows.tile([P, 1], f32, tag="alpha_pp", bufs=1)
            nc.vector.reciprocal(alpha_pp, statsr[:, 0:1])
            nc.vector.tensor_mul(alpha_pp, alpha_pp, statsr[:, 1:2])

            # mask = (sel & rng) * alpha, rounded once to bf16
            mask_cols = rows.tile([P, CK], f32, tag="mask_cols", bufs=1)
            nc.vector.scalar_tensor_tensor(
                mask_cols, gcnt_cols, float(NKEEP), rng_cols,
                op0=ALU.is_lt, op1=ALU.mult,
            )
            maskb_cols = rows.tile([P, CK], bf16, tag="maskb_cols", bufs=1)
            nc.vector.tensor_scalar(
                maskb_cols, mask_cols, alpha_pp, None, op0=ALU.mult
            )
            maskb_row = rows.tile([1, C], bf16, tag="maskb_row", bufs=1)
            nc.gpsimd.dma_start(out=maskb_row, in_=maskb_cols)
            mask_bc = mkpool.tile([P, C], bf16, tag="mask_bc")
            nc.gpsimd.partition_broadcast(mask_bc, maskb_row)
            return mask_bc

        def pass2(s, xbs, mask_bc):
            # ---- pass 2: out = xb * mask, in place (bf16 2x mode) ----
            for k in range(nk):
                for t in range(CT):
                    nc.vector.tensor_mul(
                        xbs[k][:, t * C:(t + 1) * C],
                        xbs[k][:, t * C:(t + 1) * C],
                        mask_bc,
                    )
                nc.gpsimd.dma_start(out=ok[s, k], in_=xbs[k])

        # Software-pipelined emission: mid(s) -> pass1(s+1) -> pass2(s).
        # Keeping pass1(s+1)'s DVE pair-adds AHEAD of pass2(s)'s 32
        # multiplies in the DVE program order lets the f32 chunk pool
        # recycle during mid+pass2, so the SP load stream never stalls
        # at sample boundaries (this was a ~25us/sample DMA valley).
        st = pass1(0)
        for s in range(ns):
            mask_bc = mid(s, st[0], st[1], st[2])
            xbs = st[3]
            if s + 1 < ns:
                st = pass1(s + 1)
            pass2(s, xbs, mask_bc)


def build_nc(ns=NS, hw=HW, xbufs=4, xbbufs=31):
    nc = bacc.Bacc(
        "TRN2", target_bir_lowering=False, debug=False, num_devices=NCORES
    )
    x = nc.dram_tensor("x", [ns, hw, C], f32, kind="ExternalInput").ap()
    r = nc.dram_tensor("r", [ns, C], f32, kind="ExternalInput").ap()
    u = nc.dram_tensor("u", [ns, C], f32, kind="ExternalInput").ap()
    o = nc.dram_tensor("o", [ns, hw, C], bf16, kind="ExternalOutput").ap()
    with tile.TileContext(nc) as tc:
        emit(tc, o, x, r, u, ns, hw, xbufs, xbbufs)
    nc.compile()
    return nc


_cached_nc = None


def kernel(x, r, u):
    global _cached_nc
    if _cached_nc is None:
        _cached_nc = build_nc()
    in_maps = [
        {
            "x": np.ascontiguousarray(x[i * NS:(i + 1) * NS], dtype=np.float32),
            "r": np.ascontiguousarray(r[i * NS:(i + 1) * NS], dtype=np.float32),
            "u": np.ascontiguousarray(u[i * NS:(i + 1) * NS], dtype=np.float32),
        }
        for i in range(NCORES)
    ]
    res = run_bass_kernel_spmd(_cached_nc, in_maps, list(range(NCORES))).results
    return np.concatenate(
        [np.asarray(res[i]["o"], dtype=np.float32) for i in range(NCORES)], axis=0
    )


# revision 24
# speedup vs baseline: 1.0867x; 1.0257x over previous
"""Trainium2 Bass kernel for DropChannel (topk channel masking).

Math (per sample):
    score_c = sum_hw x[hw, c]                        (1/HW cancels in the key)
    lk_c    = ln(r_c) / score_c                      (log key; order-preserving)
    gcnt_i  = #{c : lk_c > lk_i}                     (strictly-greater count)
    sel_i   = gcnt_i < C - M                         (matches thr = sort(key)[C-M]; sel = key >= thr,
                                                      including tie behaviour)
    alpha   = sum(S) / sum(S * sel)
    out     = x * (sel & (u < P)) * alpha

Sharding: pure data parallel, N=32 samples -> 8 cores x 4 samples.

v2 schedule (per core, 4 samples, each [4096, 1024] f32):
  - x streamed in 2-tile f32 chunks [128, 2048] on the SP HWDGE queue;
    each chunk feeds (a) fp32 PE ones-matmuls accumulating the channel
    score sums in PSUM and (b) an ACT-engine Copy that materializes a
    resident bf16 replica.  The f32 buffer then recycles immediately,
    so prefetch depth is bounded by the cheap bf16 pool, not f32.
  - output is written to HBM in bf16 (rel-err ~2^-9 per rounding, far
    inside the 2e-2 gate) halving store traffic: 96 MiB/core total.
  - gcnt by free-axis accumulation: tensor_scalar(is_gt) with accum_out
    against a broadcast lk row, all in (p k) column layout; no PE
    matmuls and no [128,512] compare tensors to store.
  - alpha via gpsimd.partition_all_reduce (no row-layout round trip).
  - pass 2 is a bf16 TensorTensor (DVE 2x mode) against the
    alpha-scaled broadcast mask, in place in the bf16 replica.
  - queues: loads own the SP HWDGE queue; bf16 conversions + fast mid
    DMAs ride ACT; the mask row DMA (which waits on the compare
    chain), the broadcasts, and all output stores ride the gpsimd
    SWDGE queue so they never block the ACT conversion stream or the
    SP prefetch stream.
"""

import numpy as np
from contextlib import ExitStack

import concourse.bacc as bacc
import concourse.tile as tile
from concourse import mybir
from concourse import bass_isa
from concourse.bass_utils import run_bass_kernel_spmd

N, HW, C = 32, 4096, 1024
NCORES = 8
NS = N // NCORES          # samples per core
P = 128                   # partitions
CK = C // P               # 8 channels per partition in (p k) layout
NKEEP = C - int(0.5 * C)  # gcnt threshold: keep rows with gcnt < 512
PKEEP = 0.9
HALF = 512                # matmul free-dim limit (one PSUM bank)
CT = 2                    # tiles per DMA chunk
CW = CT * C               # chunk free width (f32 elems)

f32 = mybir.dt.float32
bf16 = mybir.dt.bfloat16
ALU = mybir.AluOpType
ACTF = mybir.ActivationFunctionType
AXIS = mybir.AxisListType

# The score sums stay fp32 end to end: the selection boundary sits only
# ~2e-5 (relative) from the threshold, so bf16 anywhere in the score /
# log-key path flips channels.  The data path (x replica, mask, output)
# is bf16: three roundings of <=2^-9 each, ~0.6% worst case.


def emit(tc, o, x, r, u, ns, hw, xbufs, xbbufs):
    nc = tc.nc
    nt = hw // P              # 32 tiles
    nk = nt // CT             # chunks per sample
    # chunk view: hw = (k*CT + t)*P + p ; free dims [p][t][c]
    xk = x.rearrange("s (k t p) c -> s k p t c", t=CT, p=P)
    ok = o.rearrange("s (k t p) c -> s k p t c", t=CT, p=P)
    rck = r.rearrange("s (p k) -> s p k", k=CK)
    uck = u.rearrange("s (p k) -> s p k", k=CK)

    with ExitStack() as ctx:
        xpool = ctx.enter_context(tc.tile_pool(name="xpool", bufs=xbufs))
        xbpool = ctx.enter_context(tc.tile_pool(name="xbpool", bufs=xbbufs))
        tqpool = ctx.enter_context(tc.tile_pool(name="tqpool", bufs=3))
        bcpool = ctx.enter_context(tc.tile_pool(name="bcpool", bufs=2))
        mkpool = ctx.enter_context(tc.tile_pool(name="mkpool", bufs=2))
        scpool = ctx.enter_context(tc.tile_pool(name="scpool", bufs=2))
        rows = ctx.enter_context(tc.tile_pool(name="rows", bufs=2))
        consts = ctx.enter_context(tc.tile_pool(name="consts", bufs=1))
        ps_s = ctx.enter_context(tc.tile_pool(name="ps_s", bufs=2, space="PSUM"))

        ones_col = consts.tile([P, 1], f32)
        nc.vector.memset(ones_col, 1.0)
        ones_bc = consts.tile([P, C], bf16)
        nc.vector.memset(ones_bc, 1.0)

        def pass1(s):
            """Stream chunks of sample s: PE accumulates score sums in
            PSUM, ACT writes the resident bf16 replica, DVE pair-adds
            (fp32 PE matmuls lower to 2 half-speed passes each, so
            halving the matmul count is worth one f32 add per chunk).
            Also the x-independent precompute: ln(r), bernoulli gate."""
            lnr_row = rows.tile([1, C], f32, tag="lnr_row")
            nc.sync.dma_start(out=lnr_row, in_=r[s:s + 1, :])
            nc.scalar.activation(lnr_row, lnr_row, ACTF.Ln)
            rng_cols = rows.tile([P, CK], f32, tag="rng_cols")
            nc.sync.dma_start(out=rng_cols, in_=uck[s])
            nc.vector.tensor_scalar(
                rng_cols, rng_cols, PKEEP, None, op0=ALU.is_lt
            )
            ps_score = ps_s.tile([1, C], f32, tag="ps_score")
            xbs = []
            for k in range(nk):
                xc = xpool.tile([P, CW], f32, tag="xc")
                # sample 0 has no store traffic yet: split its load
                # triggers across both HWDGE queues for a faster ramp
                if s == 0 and k % 2 == 1:
                    nc.scalar.dma_start(out=xc, in_=xk[s, k])
                else:
                    nc.sync.dma_start(out=xc, in_=xk[s, k])
                xb = xbpool.tile([P, CW], bf16, tag="xb")
                nc.scalar.copy(xb, xc)
                xbs.append(xb)
                acc = tqpool.tile([P, C], f32, tag="acc")
                nc.vector.tensor_add(acc, xc[:, 0:C], xc[:, C:CW])
                for h in range(2):
                    nc.tensor.matmul(
                        ps_score[:, h * HALF:(h + 1) * HALF],
                        lhsT=ones_col,
                        rhs=acc[:, h * HALF:(h + 1) * HALF],
                        start=(k == 0),
                        stop=(k == nk - 1),
                    )
            return lnr_row, rng_cols, ps_score, xbs

        def mid(s, lnr_row, rng_cols, ps_score):
            # ---- mid: selection mask + alpha ----
            # log-key in row space: the ACT engine copies the PSUM score
            # row while the DVE drains pass2(s-1), and the serial small-
            # DMA chain shrinks to one hop (lk_row -> lk_cols) before
            # the compares; s_cols is only needed by the alpha chain so
            # its DMA hides under the compares.
            s_row = rows.tile([1, C], f32, tag="s_row", bufs=1)
            nc.scalar.copy(s_row, ps_score)
            recip_row = rows.tile([1, C], f32, tag="recip_row", bufs=1)
            nc.vector.reciprocal(recip_row, s_row)
            lk_row = rows.tile([1, C], f32, tag="lk_row", bufs=1)
            nc.vector.tensor_mul(lk_row, lnr_row, recip_row)
            lk_cols = rows.tile([P, CK], f32, tag="lk_cols", bufs=1)
            nc.scalar.dma_start(out=lk_cols, in_=lk_row)
            s_cols = rows.tile([P, CK], f32, tag="s_cols", bufs=1)
            nc.scalar.dma_start(out=s_cols, in_=s_row)

            # gcnt_i = #{c : lk_c > lk_i}: compare the full broadcast-lk
            # row against per-partition lk scalars, accumulating along
            # the free axis -> gcnt directly in (p k) layout.
            # accum_out via STT: tensor_scalar's accum path writes zeros
            # on hardware, STT's is the baseline-proven one.
            gcnt_cols = rows.tile([P, CK], f32, tag="gcnt_cols", bufs=1)
            scrap = scpool.tile([P, C], bf16, tag="scrap", bufs=2)
            b_bc = bcpool.tile([P, C], f32, tag="b_bc")
            nc.gpsimd.partition_broadcast(b_bc, lk_row)
            for q in range(CK):
                nc.vector.scalar_tensor_tensor(
                    scrap, b_bc, lk_cols[:, q:q + 1], ones_bc,
                    op0=ALU.is_gt, op1=ALU.mult,
                    accum_out=gcnt_cols[:, q:q + 1],
                )

            # alpha = sum(S) / sum(S * sel), all-reduced across partitions
            stats = rows.tile([P, 2], f32, tag="stats", bufs=1)
            sel8 = rows.tile([P, CK], f32, tag="sel8", bufs=1)
            nc.vector.scalar_tensor_tensor(
                sel8, gcnt_cols, float(NKEEP), s_cols,
                op0=ALU.is_lt, op1=ALU.mult, accum_out=stats[:, 0:1],
            )
            nc.vector.tensor_reduce(stats[:, 1:2], s_cols, axis=AXIS.X, op=ALU.add)
            statsr = rows.tile([P, 2], f32, tag="statsr", bufs=1)
            nc.gpsimd.partition_all_reduce(
                statsr, stats, channels=P, reduce_op=bass_isa.ReduceOp.add
            )
            alpha_pp = rows.tile([P, 1], f32, tag="alpha_pp", bufs=1)
            nc.vector.reciprocal(alpha_pp, statsr[:, 0:1])
            nc.vector.tensor_mul(alpha_pp, alpha_pp, statsr[:, 1:2])

            # mask = (sel & rng) * alpha, rounded once to bf16
            mask_cols = rows.tile([P, CK], f32, tag="mask_cols", bufs=1)
            nc.vector.scalar_tensor_tensor(
                mask_cols, gcnt_cols, float(NKEEP), rng_cols,
                op0=ALU.is_lt, op1=ALU.mult,
            )
            maskb_cols = rows.tile([P, CK], bf16, tag="maskb_cols", bufs=1)
            nc.vector.tensor_scalar(
                maskb_cols, mask_cols, alpha_pp, None, op0=ALU.mult
            )
            maskb_row = rows.tile([1, C], bf16, tag="maskb_row", bufs=1)
            nc.gpsimd.dma_start(out=maskb_row, in_=maskb_cols)
            mask_bc = mkpool.tile([P, C], bf16, tag="mask_bc")
            nc.gpsimd.partition_broadcast(mask_bc, maskb_row)
            return mask_bc

        def pass2(s, xbs, mask_bc):
            # ---- pass 2: out = xb * mask, in place (bf16 2x mode),
            # one chunk-wide TT via a stride-0 broadcast mask AP ----
            mask3 = mask_bc.rearrange("p (o c) -> p o c", o=1).broadcast_to(
                [P, CT, C]
            )
            for k in range(nk):
                xb3 = xbs[k].rearrange("p (t c) -> p t c", t=CT)
                nc.vector.tensor_mul(xb3, xb3, mask3)
                nc.gpsimd.dma_start(out=ok[s, k], in_=xbs[k])

        # Software-pipelined emission: mid(s) -> pass1(s+1) -> pass2(s).
        # Keeping pass1(s+1)'s DVE pair-adds AHEAD of pass2(s)'s 32
        # multiplies in the DVE program order lets the f32 chunk pool
        # recycle during mid+pass2, so the SP load stream never stalls
        # at sample boundaries (this was a ~25us/sample DMA valley).
        st = pass1(0)
        for s in range(ns):
            mask_bc = mid(s, st[0], st[1], st[2])
            xbs = st[3]
            if s + 1 < ns:
                st = pass1(s + 1)
            pass2(s, xbs, mask_bc)


def build_nc(ns=NS, hw=HW, xbufs=4, xbbufs=30):
    nc = bacc.Bacc(
        "TRN2", target_bir_lowering=False, debug=False, num_devices=NCORES
    )
    x = nc.dram_tensor("x", [ns, hw, C], f32, kind="ExternalInput").ap()
    r = nc.dram_tensor("r", [ns, C], f32, kind="ExternalInput").ap()
    u = nc.dram_tensor("u", [ns, C], f32, kind="ExternalInput").ap()
    o = nc.dram_tensor("o", [ns, hw, C], bf16, kind="ExternalOutput").ap()
    with tile.TileContext(nc) as tc:
        emit(tc, o, x, r, u, ns, hw, xbufs, xbbufs)
    nc.compile()
    return nc


_cached_nc = None


def kernel(x, r, u):
    global _cached_nc
    if _cached_nc is None:
        _cached_nc = build_nc()
    in_maps = [
        {
            "x": np.ascontiguousarray(x[i * NS:(i + 1) * NS], dtype=np.float32),
            "r": np.ascontiguousarray(r[i * NS:(i + 1) * NS], dtype=np.float32),
            "u": np.ascontiguousarray(u[i * NS:(i + 1) * NS], dtype=np.float32),
        }
        for i in range(NCORES)
    ]
    res = run_bass_kernel_spmd(_cached_nc, in_maps, list(range(NCORES))).results
    return np.concatenate(
        [np.asarray(res[i]["o"], dtype=np.float32) for i in range(NCORES)], axis=0
    )
